# revision 3
# baseline (speedup 1.0000x reference)
"""Trainium2 Bass kernel v8 (v4 + warmup gather + smallest-call-first): embedding gather + 2-layer MLP, data-parallel 8 cores.

vs v3: 16 gather calls of 1152 rows (earlier first data, 4-queue-parallel
descriptor gen), and elementwise ops fused to 1024 columns (2 chunks) to
halve ACT/DVE instruction count. PSUM: xtr-pair bf16 1 bank x2bufs,
h0/h1/out pairs f32 2 banks x1 buf = 8 banks exactly.
"""

import numpy as np
from contextlib import ExitStack

import concourse.bass as bass
import concourse.bacc as bacc
import concourse.tile as tile
from concourse import mybir
from concourse.bass_utils import run_bass_kernel_spmd
from concourse.masks import make_identity
import ml_dtypes

F32 = mybir.dt.float32
BF16 = mybir.dt.bfloat16
I16 = mybir.dt.int16

N_CORES = 8
B = 131072
FEAT = 128
NHID = 256
NOUT = 86
NROWS = 500000 * 4
P = 128

SEGBITS = 15
SEGSZ = 1 << SEGBITS
NSEG = (NROWS + SEGSZ - 1) // SEGSZ
NROWS_PAD = NSEG * SEGSZ

NCALL = 16                 # dma_gather calls per core
NJG = 9                    # j-columns per call (1152 rows)
CAP = NJG * P              # max rows per call / segment piece
NJ = NCALL * NJG           # 144 j-columns per core
BCP = NJ * P               # 18432 row slots per core
CHUNK_J = 4                # j-columns per MLP chunk (512 batch)
NCHUNK = NJ // CHUNK_J     # 36
NPAIR = NCHUNK // 2        # 18 elementwise pairs


def _build_program():
    nc = bacc.Bacc("TRN2", num_devices=N_CORES, num_swdge_queues=4,
                   dynamic_dma_scratch_size=65536)

    segs = [
        nc.dram_tensor(f"seg{k}", [SEGSZ, FEAT], BF16, kind="ExternalInput").ap()
        for k in range(NCALL)
    ]
    idxs = nc.dram_tensor("idxs", [P, NCALL, CAP // 16], I16, kind="ExternalInput").ap()
    w1t = nc.dram_tensor("w1t", [FEAT, NHID], BF16, kind="ExternalInput").ap()
    w2t = nc.dram_tensor("w2t", [NHID, NOUT], BF16, kind="ExternalInput").ap()
    b1v = nc.dram_tensor("b1v", [NHID], F32, kind="ExternalInput").ap()
    b2v = nc.dram_tensor("b2v", [NOUT], F32, kind="ExternalInput").ap()
    outT = nc.dram_tensor("outT", [NOUT, BCP], BF16, kind="ExternalOutput").ap()

    with tile.TileContext(nc) as tc, ExitStack() as ctx:
        const = ctx.enter_context(tc.tile_pool(name="const", bufs=1))
        gpool = ctx.enter_context(tc.tile_pool(name="gather", bufs=NCALL))
        xpool = ctx.enter_context(tc.tile_pool(name="xt", bufs=3))
        hpool = ctx.enter_context(tc.tile_pool(name="ht", bufs=2))
        opool = ctx.enter_context(tc.tile_pool(name="ot", bufs=3))
        psum = ctx.enter_context(tc.tile_pool(name="psum", bufs=1, space="PSUM"))

        idt = const.tile([P, P], BF16)
        make_identity(nc, idt[:])

        w1t_t = const.tile([FEAT, NHID], BF16)
        nc.sync.dma_start(w1t_t[:], w1t[:])
        w2t_t = const.tile([P, NHID // P, NOUT], BF16)
        nc.sync.dma_start(w2t_t[:], w2t.rearrange("(k p) n -> p k n", p=P))
        b1_t = const.tile([P, NHID // P], F32)
        nc.sync.dma_start(b1_t[:], b1v.rearrange("(k p) -> p k", p=P))
        b2_t = const.tile([NOUT, 1], F32)
        nc.sync.dma_start(b2_t[:], b2v.rearrange("(n one) -> n one", one=1))
        idx_t = const.tile([P, NCALL, CAP // 16], I16)
        nc.sync.dma_start(idx_t[:], idxs[:])

        # 16-idx warmups, one per queue: forces the mlp Q7 library load +
        # first-call IRAM overhead to happen during the const DMA loads, and
        # keeps per-queue gather counts uniform (Tile locks DMASW sem lanes
        # per SWDGE queue).
        for wq in range(4):
            warm = const.tile([P, 1, FEAT], BF16, tag=f"warm{wq}")
            nc.gpsimd.dma_gather(
                warm[:, :, :], segs[0][:], idx_t[:, 0, 0:1], 16, 16,
                elem_size=FEAT, queue_num=wq, single_packet=False,
            )
        gts = []
        for k in range(NCALL):
            gt = gpool.tile([P, NJG, FEAT], BF16, tag="gt")
            nc.gpsimd.dma_gather(
                # queues 0-3 round-robin: descriptor gen is gated per queue to
                # one Q7 pair (~9.3 ns/idx serial), so 4 queues generate 4-way
                # in parallel. single_packet=False: the coalesced stream would
                # exceed the 64-desc/16KB SDMA packet limit and wedge.
                gt[:, :, :], segs[k][:], idx_t[:, k, :], CAP, CAP,
                elem_size=FEAT, queue_num=k % 4,
                single_packet=False,
            )
            gts.append(gt)

        W = 2 * CHUNK_J * P  # 1024: elementwise/pair width
        for pc in range(NPAIR):
            bcol = pc * W

            # transpose 8x [128b,128f] -> one bf16 PSUM bank [128, 1024]
            xtr = psum.tile([P, 2 * CHUNK_J, P], BF16, tag="xtr")
            for i in range(2 * CHUNK_J):
                j = pc * 2 * CHUNK_J + i
                nc.tensor.transpose(
                    out=xtr[:, i, :],
                    in_=gts[j // NJG][:, j % NJG, :],
                    identity=idt[:],
                )
            xt = xpool.tile([P, W], BF16)
            nc.vector.tensor_copy(out=xt[:], in_=xtr.rearrange("p i f -> p (i f)"))

            # lin1: two 512-col matmuls per hid half into a 2-bank pair tile,
            # one 1024-col relu+bias on ACT per half
            ht = hpool.tile([P, NHID // P, W], BF16)
            for k in range(NHID // P):
                hp = psum.tile([P, 2, CHUNK_J * P], F32, tag=f"h{k}")
                for c in range(2):
                    nc.tensor.matmul(
                        out=hp[:, c, :],
                        lhsT=w1t_t[:, k * P:(k + 1) * P],
                        rhs=xt[:, c * CHUNK_J * P:(c + 1) * CHUNK_J * P],
                        start=True,
                        stop=True,
                    )
                nc.scalar.activation(
                    out=ht[:, k, :], in_=hp.rearrange("p c n -> p (c n)"),
                    func=mybir.ActivationFunctionType.Relu,
                    bias=b1_t[:, k:k + 1],
                )

            # lin2: 2 chunks x 2 k-tiles into a 2-bank pair tile, one
            # 1024-col bias+relu on DVE
            op_ = psum.tile([NOUT, 2, CHUNK_J * P], F32, tag="ot")
            for c in range(2):
                for k in range(NHID // P):
                    nc.tensor.matmul(
                        out=op_[:, c, :],
                        lhsT=w2t_t[:, k, :],
                        rhs=ht[:, k, c * CHUNK_J * P:(c + 1) * CHUNK_J * P],
                        start=(k == 0),
                        stop=(k == NHID // P - 1),
                    )
            ot = opool.tile([NOUT, W], BF16)
            nc.vector.tensor_scalar(
                out=ot[:], in0=op_.rearrange("p c n -> p (c n)"),
                scalar1=b2_t[:], scalar2=0.0,
                op0=mybir.AluOpType.add, op1=mybir.AluOpType.max,
            )
            nc.sync.dma_start(outT[:, bcol:bcol + W], ot[:])

    nc.compile()
    return nc


TRACE = False
RUN_KWARGS = None
LAST = None

_COLMAP = None


def _colmap():
    global _COLMAP
    if _COLMAP is None:
        k = np.arange(NCALL)[:, None]
        i = np.arange(CAP)[None, :]
        j = k * NJG + i // P
        p = i % P
        _COLMAP = (j // CHUNK_J) * (CHUNK_J * P) + (j % CHUNK_J) * P + p
    return _COLMAP


def _plan(flat):
    order = np.argsort(flat, kind="stable")
    sf = flat[order]
    seg_of = sf >> SEGBITS
    pieces = []
    for s in np.unique(seg_of):
        lo, hi = np.searchsorted(seg_of, [s, s + 1])
        for a in range(lo, hi, CAP):
            e = min(a + CAP, hi)
            pieces.append((e - a, int(s), a, e))
    assert len(pieces) <= N_CORES * NCALL, f"{len(pieces)} pieces > {N_CORES * NCALL}"
    pieces.sort(reverse=True)
    bins = [[] for _ in range(N_CORES)]
    loads = [0] * N_CORES
    for pc in pieces:
        b = min((bb for bb in range(N_CORES) if len(bins[bb]) < NCALL),
                key=lambda bb: loads[bb])
        bins[b].append(pc)
        loads[b] += pc[0]
    for b in range(N_CORES):
        bins[b].sort()  # smallest call first: call 0 gates compute start

    plans = []
    for b in range(N_CORES):
        seg_ids = np.zeros(NCALL, np.int64)
        idx16 = np.zeros((NCALL, CAP), np.int16)
        counts = np.zeros(NCALL, np.int64)
        pos = np.full((NCALL, CAP), -1, np.int64)
        for k, (n, s, a, e) in enumerate(bins[b]):
            seg_ids[k] = s
            idx16[k, :n] = (sf[a:e] - (s << SEGBITS)).astype(np.int16)
            counts[k] = n
            pos[k, :n] = order[a:e]
        plans.append((seg_ids, idx16, counts, pos))
    return plans


def kernel(entity_embedding, w1, b1, w2, b2, idx0, idx1):
    table = np.zeros((NROWS_PAD, FEAT), dtype=ml_dtypes.bfloat16)
    table[:NROWS] = np.asarray(entity_embedding, dtype=np.float32).reshape(
        NROWS, FEAT).astype(ml_dtypes.bfloat16)
    flat_idx = (np.asarray(idx0, dtype=np.int64) * 4
                + np.asarray(idx1, dtype=np.int64)).astype(np.int32)
    w1t = np.ascontiguousarray(
        np.asarray(w1, dtype=np.float32).T).astype(ml_dtypes.bfloat16)
    w2t = np.ascontiguousarray(
        np.asarray(w2, dtype=np.float32).T).astype(ml_dtypes.bfloat16)
    b1v = np.ascontiguousarray(np.asarray(b1, dtype=np.float32))
    b2v = np.ascontiguousarray(np.asarray(b2, dtype=np.float32))

    plans = _plan(flat_idx)
    in_maps = []
    for core in range(N_CORES):
        seg_ids, idx16, counts, pos = plans[core]
        wrap = idx16.reshape(NCALL, CAP // 16, 16).transpose(2, 0, 1)
        idx_tile = np.ascontiguousarray(np.tile(wrap, (8, 1, 1)))
        m = {"idxs": idx_tile, "w1t": w1t, "w2t": w2t, "b1v": b1v, "b2v": b2v}
        for k in range(NCALL):
            m[f"seg{k}"] = table[seg_ids[k] * SEGSZ:(seg_ids[k] + 1) * SEGSZ]
        in_maps.append(m)

    nc = _build_program()
    global LAST
    res = run_bass_kernel_spmd(
        nc, in_maps, core_ids=list(range(N_CORES)), trace=TRACE,
        **(RUN_KWARGS or {}),
    )
    LAST = res
    out = np.empty((B, NOUT), dtype=np.float32)
    cm = _colmap()
    for core in range(N_CORES):
        _, _, counts, pos = plans[core]
        rowsT = res.results[core]["outT"]
        for k in range(NCALL):
            n = counts[k]
            if n:
                out[pos[k, :n]] = rowsT[:, cm[k, :n]].T.astype(np.float32)
    return out
